# revision 18
# baseline (speedup 1.0000x reference)
"""Trainium2 Bass kernel for a tanh-RNN:
    xp = x @ W_ih + b_hh
    h_t = tanh(xp_t + h_{t-1} @ W_hh)
    y   = hs @ W_ho + b_ho

Sharding: pure data-parallel over batch across 8 NeuronCores (weights
replicated, 32 batch rows per core, zero collectives).

Per-core layout is fully transposed ("feature-on-partition") so the
sequential recurrence never needs a per-step transpose:
  - x^T  [512,  BT]  via PE transposes (phase 1)
  - xp^T [1024, BT] = W_ih^T @ x^T        (phase 1, bf16 matmul)
  - h^T state tiles [128(h) x 32(b)] bf16; step t: 8x8 matmuls with
    W_hh tiles stationary (bf16 -> fast weight load), h^T moving
  - logits^T = W_ho^T @ hs^T, then PE un-transpose  (phase 3)

BT rows are time-major (bt = t*32 + b) so each step's xp_t^T / h_t^T is
a contiguous 32-column block. The host wrapper permutes in/out.
"""

from contextlib import ExitStack

import numpy as np

import concourse.bass as bass
import concourse.bacc as bacc
import concourse.mybir as mybir
import concourse.tile as tile
from concourse.bass_utils import run_bass_kernel_spmd
from concourse.masks import make_identity

DT = mybir.dt
AFT = mybir.ActivationFunctionType

B, T, DI, DH, DO = 256, 128, 512, 1024, 50
NCORES = 8
BL = B // NCORES  # 32 batch rows per core
KI = DI // 128    # 4  k-tiles of W_ih
KH = DH // 128    # 8  k-tiles / n-tiles of W_hh


def build_module(t_steps: int = T, reps: int = 1) -> bass.Bass:
    bt = t_steps * BL  # rows per core, time-major
    ch = min(512, bt)  # moving-dim chunk for the big GEMMs

    nc = bacc.Bacc("TRN2", target_bir_lowering=False, debug=False)
    x = nc.declare_dram_parameter("x", [bt, DI], DT.float32, isOutput=False)
    wih = nc.declare_dram_parameter("W_ih", [DI, DH], DT.float32, isOutput=False)
    whh = nc.declare_dram_parameter("W_hh", [DH, DH], DT.float32, isOutput=False)
    bhh = nc.declare_dram_parameter("b_hh", [DH, 1], DT.float32, isOutput=False)
    who = nc.declare_dram_parameter("W_ho", [DH, DO], DT.float32, isOutput=False)
    bho = nc.declare_dram_parameter("b_ho", [DO, 1], DT.float32, isOutput=False)
    y = nc.declare_dram_parameter("y", [bt, DO], DT.float32, isOutput=True)

    with tile.TileContext(nc) as tc, ExitStack() as outer:
        if reps > 1:
            # timing-only: repeat the whole kernel on-device so host
            # dispatch/transfer overhead cancels in a wall-time delta
            outer.enter_context(tc.For_i(0, reps, 1))
        ctx = outer.enter_context(ExitStack())
        # All DMA-written buffers live in virgin SBUF space (never-reused):
        # the walrus DIRECT2D lowering allows at most ONE embedded sync wait
        # per DMA, so DMAs must never inherit space-reuse deps.
        const = ctx.enter_context(tc.tile_pool(name="const", bufs=1))
        ident = const.tile([128, 128], DT.bfloat16)
        make_identity(nc, ident[:])
        ident32 = const.tile([128, 128], DT.float32)
        make_identity(nc, ident32[:])
        bhh_sb = const.tile([128, KH], DT.float32)
        for n0 in range(KH):
            nc.gpsimd.dma_start(bhh_sb[:, n0 : n0 + 1], bhh[n0 * 128 : (n0 + 1) * 128, :])
        bho_sb = const.tile([DO, 1], DT.float32)
        nc.gpsimd.dma_start(bho_sb[:], bho[:])

        # weight loads: gpsimd DMA casts fp32 -> bf16 in flight, write-once
        whh_sb = ctx.enter_context(tc.tile_pool(name="whh", bufs=1)).tile(
            [128, KH, DH], DT.bfloat16
        )
        for k0 in range(KH):
            nc.gpsimd.dma_start(whh_sb[:, k0, :], whh[k0 * 128 : (k0 + 1) * 128, :])
        wih_sb = ctx.enter_context(tc.tile_pool(name="wihp", bufs=1)).tile(
            [128, KI, DH], DT.bfloat16
        )
        for k0 in range(KI):
            nc.gpsimd.dma_start(wih_sb[:, k0, :], wih[k0 * 128 : (k0 + 1) * 128, :])
        who_sb = ctx.enter_context(tc.tile_pool(name="whop", bufs=1)).tile(
            [128, KH, DO], DT.bfloat16
        )
        for k0 in range(KH):
            nc.gpsimd.dma_start(who_sb[:, k0, :], who[k0 * 128 : (k0 + 1) * 128, :])
        xbf = ctx.enter_context(tc.tile_pool(name="xbf", bufs=1)).tile(
            [128, bt // 128, DI], DT.bfloat16
        )
        xpT = ctx.enter_context(tc.tile_pool(name="xpT", bufs=1)).tile(
            [128, KH, bt], DT.bfloat16
        )

        # ---- phase 1: x^T then xp^T = W_ih^T @ x^T (+ b_hh) ----
        with tc.tile_pool(name="xt", bufs=1) as xtp:
            xT = xtp.tile([128, KI, bt], DT.bfloat16)
            with tc.tile_pool(name="tps", bufs=4, space="PSUM") as tps:
                for bt0 in range(bt // 128):
                    nc.gpsimd.dma_start(
                        xbf[:, bt0, :], x[bt0 * 128 : (bt0 + 1) * 128, :]
                    )
                    for k0 in range(KI):
                        ps = tps.tile([128, 128], DT.bfloat16, tag="tps")
                        nc.tensor.transpose(
                            ps[:], xbf[:, bt0, k0 * 128 : (k0 + 1) * 128], ident[:]
                        )
                        nc.vector.tensor_copy(
                            xT[:, k0, bt0 * 128 : (bt0 + 1) * 128], ps[:]
                        )
            with tc.tile_pool(name="gps", bufs=4, space="PSUM") as gps:
                for btc in range(bt // ch):
                    for n0 in range(KH):
                        ps = gps.tile([128, ch], DT.float32, tag="gps")
                        for k0 in range(KI):
                            nc.tensor.matmul(
                                ps[:],
                                wih_sb[:, k0, n0 * 128 : (n0 + 1) * 128],
                                xT[:, k0, btc * ch : (btc + 1) * ch],
                                start=(k0 == 0),
                                stop=(k0 == KI - 1),
                            )
                        nc.scalar.activation(
                            xpT[:, n0, btc * ch : (btc + 1) * ch],
                            ps[:],
                            AFT.Identity,
                            bias=bhh_sb[:, n0 : n0 + 1],
                        )

        # recurrence state: engine(ACT)-written, may reuse xT's freed space
        hsT = ctx.enter_context(tc.tile_pool(name="hsT", bufs=1)).tile(
            [128, KH, bt], DT.bfloat16
        )

        # ---- phase 2: recurrence (hybrid) ----
        # B-part (n0 < NB): W_hh tile stationary (LDW port), h^T moving.
        # A-part (n0 >= NB): h^T stationary, W_hh columns moving (MM port),
        # computing h_new rows [32, NA*128]; un-transposed back on the PE.
        # Splitting W_hh across both PE input ports beats the pure
        # stationary form, which is LDW-port-bound at ~53ns/tile.
        NB = 5 if t_steps > 1 else KH
        NA = KH - NB
        with (
            tc.tile_pool(name="rps", bufs=4, space="PSUM") as rps,
            tc.tile_pool(name="aps", bufs=2, space="PSUM") as aps,
            tc.tile_pool(name="tp2", bufs=2, space="PSUM") as tp2,
            tc.tile_pool(name="asb", bufs=2) as asbp,
        ):
            for n0 in range(KH):
                nc.scalar.activation(
                    hsT[:, n0, 0:BL], xpT[:, n0, 0:BL], AFT.Tanh
                )
            for t in range(1, t_steps):
                for n0 in range(NB):
                    ps = rps.tile([128, BL], DT.float32, tag="rps")
                    for k0 in range(KH):
                        nc.tensor.matmul(
                            ps[:],
                            whh_sb[:, k0, n0 * 128 : (n0 + 1) * 128],
                            hsT[:, k0, (t - 1) * BL : t * BL],
                            start=(k0 == 0),
                            stop=(k0 == KH - 1),
                        )
                    nc.vector.tensor_add(
                        ps[:], ps[:], xpT[:, n0, t * BL : (t + 1) * BL]
                    )
                    nc.scalar.activation(
                        hsT[:, n0, t * BL : (t + 1) * BL], ps[:], AFT.Tanh
                    )
                if NA:
                    psA = aps.tile([BL, NA * 128], DT.float32, tag="aps")
                    for k0 in range(KH):
                        nc.tensor.matmul(
                            psA[:],
                            hsT[:, k0, (t - 1) * BL : t * BL],
                            whh_sb[:, k0, NB * 128 :],
                            start=(k0 == 0),
                            stop=(k0 == KH - 1),
                        )
                    aSb = asbp.tile([BL, NA * 128], DT.bfloat16, tag="asb")
                    nc.vector.tensor_copy(aSb[:], psA[:])
                    for j in range(NA):
                        n0 = NB + j
                        pst = tp2.tile([128, BL], DT.bfloat16, tag="tp2")
                        nc.tensor.transpose(
                            pst[:], aSb[:, j * 128 : (j + 1) * 128], ident[:BL, :BL]
                        )
                        pre = asbp.tile([128, BL], DT.bfloat16, tag="pre")
                        nc.vector.tensor_add(
                            pre[:], pst[:], xpT[:, n0, t * BL : (t + 1) * BL]
                        )
                        nc.scalar.activation(
                            hsT[:, n0, t * BL : (t + 1) * BL], pre[:], AFT.Tanh
                        )

        # ---- phase 3: logits^T = W_ho^T @ hs^T (+ b_ho), un-transpose, store ----
        with tc.tile_pool(name="lot", bufs=1) as lotp:
            loT = lotp.tile([DO, bt], DT.bfloat16)
            with tc.tile_pool(name="ops", bufs=4, space="PSUM") as ops:
                for btc in range(bt // ch):
                    ps = ops.tile([DO, ch], DT.float32, tag="ops")
                    for k0 in range(KH):
                        nc.tensor.matmul(
                            ps[:],
                            who_sb[:, k0, :],
                            hsT[:, k0, btc * ch : (btc + 1) * ch],
                            start=(k0 == 0),
                            stop=(k0 == KH - 1),
                        )
                    nc.scalar.activation(
                        loT[:, btc * ch : (btc + 1) * ch],
                        ps[:],
                        AFT.Identity,
                        bias=bho_sb[:],
                    )
            with (
                tc.tile_pool(name="yps", bufs=4, space="PSUM") as yps,
                tc.tile_pool(name="yout", bufs=8) as youtp,
            ):
                for c in range(bt // 128):
                    ps = yps.tile([128, DO], DT.bfloat16, tag="yps")
                    nc.tensor.transpose(
                        ps[:], loT[:, c * 128 : (c + 1) * 128], ident[:DO, :DO]
                    )
                    yt = youtp.tile([128, DO], DT.float32, tag="yout")
                    nc.vector.tensor_copy(yt[:], ps[:])
                    nc.gpsimd.dma_start(y[c * 128 : (c + 1) * 128, :], yt[:])

    nc.compile()
    return nc


_MODULE_CACHE: dict = {}
_EXEC_CACHE: dict = {}


def _get_module(t_steps: int = T, reps: int = 1) -> bass.Bass:
    key = (t_steps, reps)
    if key not in _MODULE_CACHE:
        _MODULE_CACHE[key] = build_module(t_steps, reps)
    return _MODULE_CACHE[key]


def _get_executor(t_steps: int = T, repeat: int = 1):
    """Jitted SPMD executor over 8 cores. `repeat` chains the kernel
    serially on-device (output y feeds the next call's donated output
    buffer) so per-execution HW time can be measured as a wall-time
    difference without NTFF profiling."""
    key = (t_steps, repeat)
    if key in _EXEC_CACHE:
        return _EXEC_CACHE[key]
    donate = repeat == 1

    import jax
    from jax.sharding import Mesh, PartitionSpec
    from jax.experimental.shard_map import shard_map
    from concourse.bass2jax import (
        _bass_exec_p,
        install_neuronx_cc_hook,
        partition_id_tensor,
    )

    install_neuronx_cc_hook()
    nc = _get_module(t_steps, repeat)
    assert nc.dbg_addr is None
    partition_name = nc.partition_id_tensor.name if nc.partition_id_tensor else None

    in_names, out_names, out_avals = [], [], []
    for alloc in nc.m.functions[0].allocations:
        if not isinstance(alloc, mybir.MemoryLocationSet):
            continue
        name = alloc.memorylocations[0].name
        if alloc.kind == "ExternalInput":
            if name != partition_name:
                in_names.append(name)
        elif alloc.kind == "ExternalOutput":
            out_names.append(name)
            out_avals.append(
                jax.core.ShapedArray(
                    tuple(alloc.tensor_shape), mybir.dt.np(alloc.dtype)
                )
            )
    n_params = len(in_names)
    n_outs = len(out_names)
    all_in_names = tuple(
        in_names + out_names + ([partition_name] if partition_name else [])
    )

    def _call(ops):
        if partition_name:
            ops = ops + [partition_id_tensor()]
        return _bass_exec_p.bind(
            *ops,
            out_avals=tuple(out_avals),
            in_names=all_in_names,
            out_names=tuple(out_names),
            lowering_input_output_aliases=(),
            sim_require_finite=True,
            sim_require_nnan=True,
            nc=nc,
        )

    def _body(*args):
        ins = list(args[:n_params])
        outs = list(args[n_params:])
        return tuple(_call(ins + outs))

    devices = jax.devices()[:NCORES]
    mesh = Mesh(np.asarray(devices), ("core",))
    in_specs = (PartitionSpec("core"),) * (n_params + n_outs)
    out_specs = (PartitionSpec("core"),) * n_outs
    jitted = jax.jit(
        shard_map(
            _body, mesh=mesh, in_specs=in_specs, out_specs=out_specs, check_rep=False
        ),
        donate_argnums=tuple(range(n_params, n_params + n_outs)) if donate else (),
        keep_unused=True,
    )

    def prepare(in_maps):
        per_core = [[np.asarray(m[name]) for name in in_names] for m in in_maps]
        concat_in = [
            np.concatenate([per_core[c][i] for c in range(NCORES)], axis=0)
            for i in range(n_params)
        ]
        concat_zeros = [
            np.zeros((NCORES * a.shape[0], *a.shape[1:]), a.dtype) for a in out_avals
        ]
        args = concat_in + concat_zeros
        if not donate:
            # keep args on device for repeatable low-overhead calls
            from jax.sharding import NamedSharding

            sh = NamedSharding(mesh, PartitionSpec("core"))
            args = [jax.device_put(a, sh) for a in args]
        return args

    def raw_call(args):
        return jitted(*args)

    def execute(in_maps):
        out_arrs = raw_call(prepare(in_maps))
        return [
            {
                name: np.asarray(out_arrs[i]).reshape(
                    NCORES, *out_avals[i].shape
                )[c]
                for i, name in enumerate(out_names)
            }
            for c in range(NCORES)
        ]

    execute.prepare = prepare
    execute.raw_call = raw_call
    _EXEC_CACHE[key] = execute
    return execute


def _make_in_maps(x, W_ih, W_hh, b_hh, W_ho, b_ho, t_steps: int = T):
    x = np.ascontiguousarray(np.asarray(x, dtype=np.float32))
    W_ih = np.ascontiguousarray(np.asarray(W_ih, dtype=np.float32))
    W_hh = np.ascontiguousarray(np.asarray(W_hh, dtype=np.float32))
    b_hh = np.ascontiguousarray(np.asarray(b_hh, dtype=np.float32).reshape(DH, 1))
    W_ho = np.ascontiguousarray(np.asarray(W_ho, dtype=np.float32))
    b_ho = np.ascontiguousarray(np.asarray(b_ho, dtype=np.float32).reshape(DO, 1))
    in_maps = []
    for c in range(NCORES):
        # [BL, t, DI] -> time-major [t*BL, DI]
        xc = np.ascontiguousarray(
            x[c * BL : (c + 1) * BL].transpose(1, 0, 2).reshape(t_steps * BL, DI)
        )
        in_maps.append(
            {"x": xc, "W_ih": W_ih, "W_hh": W_hh, "b_hh": b_hh, "W_ho": W_ho, "b_ho": b_ho}
        )
    return in_maps


def _assemble(results, t_steps: int = T):
    outs = []
    for c in range(NCORES):
        yc = np.asarray(results[c]["y"])  # [t*BL, DO] time-major
        outs.append(yc.reshape(t_steps, BL, DO).transpose(1, 0, 2))
    return np.concatenate(outs, axis=0)


def run(x, W_ih, W_hh, b_hh, W_ho, b_ho, t_steps: int = T, repeat: int = 1):
    """Run on hardware. x: [B, t_steps, DI]. Returns [B, t_steps, DO]."""
    in_maps = _make_in_maps(x, W_ih, W_hh, b_hh, W_ho, b_ho, t_steps)
    execute = _get_executor(t_steps, repeat)
    return _assemble(execute(in_maps), t_steps)


def kernel(x, W_ih, W_hh, b_hh, W_ho, b_ho):
    return run(x, W_ih, W_hh, b_hh, W_ho, b_ho)
